# revision 1
# baseline (speedup 1.0000x reference)
"""KoLeoLoss Trainium2 kernel (nn_KoLeoLoss_73538430042938).

Math: rows are L2-normalized, so for the nearest neighbor j of row i (by max
cosine sim m_i), the pairwise distance is ||xn_i - xn_j|| = sqrt(2 - 2*m_i).
So the device kernel only needs, per row, the max off-diagonal entry of the
Gram matrix Xn @ Xn^T -- no argmax indices, no gather.

Sharding: data-parallel over batch B=32 -> 4 batches per core on 8 cores.
Each core returns the [128, 32] row-max matrix (4096 rows); the host applies
the (tiny) log/mean reduction in float64.

Device pipeline per batch b (shapes [N=1024, D=512], P=128):
  1. DMA X_b into SBUF as [128, 8, 512] (row tiles on partitions).
  2. ssq_i = sum_d x^2 via ScalarE Square+accum; norm = sqrt(ssq) (ScalarE);
     rinv = 1/norm (VectorE reciprocal). All fp32.
  3. xn = x * rinv (ScalarE Copy with per-partition scale), output bf16.
  4. XnT [512, 1024] built with PE is_transpose matmuls ([128,128] blocks,
     bf16) staged in PSUM, copied to SBUF by VectorE.
  5. G row-tile [128, 1024] = XnT.T @ XnT accumulated over 4 K-chunks in
     PSUM (fp32). Diagonal block gets -30000 added (VectorE + const mask).
  6. rowmax via VectorE reduce_max -> maxes[:, b*8+t].
"""

import sys

import numpy as np

_TRN = "/opt/trn_rl_repo"
if _TRN not in sys.path:
    sys.path.insert(0, _TRN)

B, N, D = 32, 1024, 512
NCORES = 8
BLOC = B // NCORES  # batches per core
P = 128
NT = N // P  # row tiles per batch
KC = D // P  # contraction chunks
NEG = -30000.0
EPS = 1e-8

_CACHE = {}


def build_nc():
    import concourse.bacc as bacc
    import concourse.mybir as mybir
    from concourse import masks, tile

    f32 = mybir.dt.float32
    bf16 = mybir.dt.bfloat16
    AF = mybir.ActivationFunctionType

    nc = bacc.Bacc(
        "TRN2", target_bir_lowering=False, debug=False, num_devices=NCORES
    )
    x_dram = nc.dram_tensor("x", [BLOC, N, D], f32, kind="ExternalInput")
    out_dram = nc.dram_tensor("maxes", [P, BLOC * NT], f32, kind="ExternalOutput")

    with tile.TileContext(nc) as tc:
        with (
            tc.tile_pool(name="const", bufs=1) as cpool,
            tc.tile_pool(name="xin", bufs=2) as xpool,
            tc.tile_pool(name="xn", bufs=2) as xnpool,
            tc.tile_pool(name="xt", bufs=2) as xtpool,
            tc.tile_pool(name="stats", bufs=2) as spool,
            tc.tile_pool(name="scr", bufs=2) as scpool,
            tc.tile_pool(name="outp", bufs=1) as opool,
            tc.tile_pool(name="gpsum", bufs=2, space="PSUM") as gpool,
            tc.tile_pool(name="tpsum", bufs=2, space="PSUM") as tpool,
        ):
            ident = cpool.tile([P, P], bf16)
            masks.make_identity(nc, ident[:])
            negbig = cpool.tile([P, P], f32)
            nc.gpsimd.memset(negbig[:], 0.0)
            nc.gpsimd.affine_select(
                out=negbig[:],
                in_=negbig[:],
                compare_op=mybir.AluOpType.not_equal,
                fill=NEG,
                base=0,
                pattern=[[-1, P]],
                channel_multiplier=1,
            )

            maxes = opool.tile([P, BLOC * NT], f32)
            x_r = x_dram.ap().rearrange("b (t p) d -> b p t d", p=P)

            for b in range(BLOC):
                x_all = xpool.tile([P, NT, D], f32, tag="x_all")
                nc.sync.dma_start(x_all[:], x_r[b])

                ssq = spool.tile([P, NT], f32, tag="ssq")
                for i in range(NT):
                    sq = scpool.tile([P, D], bf16, tag="sq")
                    nc.scalar.activation(
                        sq[:], x_all[:, i], AF.Square, accum_out=ssq[:, i : i + 1]
                    )
                nrm = spool.tile([P, NT], f32, tag="nrm")
                nc.scalar.activation(nrm[:], ssq[:], AF.Sqrt)
                rinv = spool.tile([P, NT], f32, tag="rinv")
                nc.vector.reciprocal(rinv[:], nrm[:])

                xn = xnpool.tile([P, NT, D], bf16, tag="xn")
                for i in range(NT):
                    nc.scalar.activation(
                        xn[:, i], x_all[:, i], AF.Copy, scale=rinv[:, i : i + 1]
                    )

                # XnT[k][:, i*P:(i+1)*P] = transpose(xn[:, i, k*P:(k+1)*P])
                xnT = xtpool.tile([P, KC, N], bf16, tag="xnT")
                for k in range(KC):
                    for h in range(2):
                        tp = tpool.tile([P, 512], bf16, tag="tp")
                        for ii in range(4):
                            i = h * 4 + ii
                            nc.tensor.matmul(
                                tp[:, ii * P : (ii + 1) * P],
                                xn[:, i, k * P : (k + 1) * P],
                                ident[:],
                                is_transpose=True,
                            )
                        nc.vector.tensor_copy(
                            xnT[:, k, h * 512 : (h + 1) * 512], tp[:]
                        )

                for t in range(NT):
                    G = gpool.tile([P, N], f32, tag="G")
                    for k in range(KC):
                        lhsT = xnT[:, k, t * P : (t + 1) * P]
                        nc.tensor.matmul(
                            G[:, 0:512],
                            lhsT,
                            xnT[:, k, 0:512],
                            start=(k == 0),
                            stop=(k == KC - 1),
                        )
                        nc.tensor.matmul(
                            G[:, 512:N],
                            lhsT,
                            xnT[:, k, 512:N],
                            start=(k == 0),
                            stop=(k == KC - 1),
                        )
                    nc.vector.tensor_add(
                        G[:, t * P : (t + 1) * P], G[:, t * P : (t + 1) * P], negbig[:]
                    )
                    nc.vector.reduce_max(
                        maxes[:, b * NT + t : b * NT + t + 1],
                        G[:, :],
                        axis=mybir.AxisListType.X,
                    )

            nc.sync.dma_start(out_dram.ap(), maxes[:])

    nc.compile()
    return nc


def get_nc():
    if "nc" not in _CACHE:
        _CACHE["nc"] = build_nc()
    return _CACHE["nc"]


def shard_inputs(sparse_feats):
    x = np.ascontiguousarray(sparse_feats, dtype=np.float32).reshape(
        NCORES, BLOC, N, D
    )
    return [{"x": x[c]} for c in range(NCORES)]


def finalize(m_all):
    """m_all: any array containing the 32768 per-row max cosine sims."""
    m = np.asarray(m_all, dtype=np.float64)
    t = np.maximum(2.0 - 2.0 * m, 0.0)
    dist = 0.5 * np.sqrt(t)
    return np.float32(-np.mean(np.log(dist + EPS)))


def run_on_hw(sparse_feats, trace=False, **kw):
    from concourse.bass_utils import run_bass_kernel_spmd

    nc = get_nc()
    res = run_bass_kernel_spmd(
        nc, shard_inputs(sparse_feats), list(range(NCORES)), trace=trace, **kw
    )
    m = np.stack([res.results[c]["maxes"] for c in range(NCORES)])
    return finalize(m), res


def kernel(sparse_feats):
    loss, _ = run_on_hw(sparse_feats)
    return loss
